# revision 6
# baseline (speedup 1.0000x reference)
"""Trainium2 Bass kernel for DeChunking EMA (lower-triangular decay matmul).

Math: out[b,i,:] = sum_{j<=i} exp(S_i - S_j) * p_j * z[b,j,:],
with S = cumsum(log(clip(1-p))). Computed chunked-scan style (Mamba-SSD):

  - L split into C=32 chunks of Q=128.
  - Intra-chunk: out_intra = W_c^T.T @ z_c with
      W_c^T[j,i] = exp(S_i - S_j + log p_j) (masked i>=j),
    where the delta matrix S_i - S_j + log p_j is produced by a single K=3
    matmul of stacked [1, -S, logp] x [S, 1, 1] operands.
  - Inter-chunk: chunk states H_c = U_c^T @ z_c (U_c[j] = exp(Send_c - S_j
    + log p_j)), carried across chunks with one [32,32] decay matmul
    (carry = M2^T @ H), then applied per chunk as a rank-1 PSUM-accumulated
    matmul out += A_c (x) carry_c.

All exp inputs are <= 0 by construction, so nothing overflows.

Sharding (8 cores, no collectives): core = (batch b in {0,1}) x (one of 4
D-blocks of 192). Each core reads z[b, :, blk] and pt[b] only.
"""

import os
import numpy as np

B, L, D = 2, 4096, 768
Q = 128
C = L // Q           # 32 chunks
ND = 4               # D blocks per batch
DBLK = D // ND       # 192
GRP = 4              # chunks per exp/mask group
NG = C // GRP        # 8 groups
NEG = -3.0e38
N_CORES = 8

_CTX = {}
LAST_EXEC_NS = None


def _build_program():
    import concourse.bacc as bacc
    import concourse.mybir as mybir
    from concourse import tile

    f32 = mybir.dt.float32
    nc = bacc.Bacc("TRN2", target_bir_lowering=False, debug=False,
                   num_devices=N_CORES)

    z_s = nc.dram_tensor("z_s", [C, Q, DBLK], f32, kind="ExternalInput")
    stackL = nc.dram_tensor("stackL", [3, L], f32, kind="ExternalInput")
    stackR = nc.dram_tensor("stackR", [3, L], f32, kind="ExternalInput")
    uexp = nc.dram_tensor("uexp", [Q, C], f32, kind="ExternalInput")
    aexp = nc.dram_tensor("aexp", [C, Q], f32, kind="ExternalInput")
    d2exp = nc.dram_tensor("d2exp", [C, C], f32, kind="ExternalInput")
    maskb = nc.dram_tensor("maskb", [Q, GRP * Q], f32, kind="ExternalInput")
    ident = nc.dram_tensor("ident", [Q, Q], f32, kind="ExternalInput")
    out_s = nc.dram_tensor("out_s", [C, Q, DBLK], f32, kind="ExternalOutput")

    Exp = mybir.ActivationFunctionType.Exp
    HD = DBLK // 2  # 96: half of a D block, so M <= 128 in the H matvecs

    with tile.TileContext(nc) as tc:
        with (
            tc.tile_pool(name="zp", bufs=C) as zp,
            tc.tile_pool(name="wp", bufs=NG) as wp,
            tc.tile_pool(name="sp", bufs=1) as sp,
            tc.tile_pool(name="op", bufs=4) as op,
            tc.tile_pool(name="dps", bufs=2, space="PSUM") as dps,
            tc.tile_pool(name="ops", bufs=2, space="PSUM") as ops,
            tc.tile_pool(name="hps", bufs=1, space="PSUM") as hps,
        ):
            # small operand loads
            sL = sp.tile([3, L], f32, tag="sL")
            nc.sync.dma_start(sL[:], stackL[:])
            sR = sp.tile([3, L], f32, tag="sR")
            nc.sync.dma_start(sR[:], stackR[:])
            ue = sp.tile([Q, C], f32, tag="ue")
            nc.sync.dma_start(ue[:], uexp[:])
            ae = sp.tile([C, Q], f32, tag="ae")
            nc.sync.dma_start(ae[:], aexp[:])
            d2 = sp.tile([C, C], f32, tag="d2")
            nc.sync.dma_start(d2[:], d2exp[:])
            mb = sp.tile([Q, GRP * Q], f32, tag="mb")
            nc.sync.dma_start(mb[:], maskb[:])
            idn = sp.tile([Q, Q], f32, tag="idn")
            nc.sync.dma_start(idn[:], ident[:])

            U = sp.tile([Q, C], f32, tag="U")
            nc.scalar.activation(U[:], ue[:], Exp)
            A = sp.tile([C, Q], f32, tag="A")
            nc.scalar.activation(A[:], ae[:], Exp)
            M2 = sp.tile([C, C], f32, tag="M2")
            nc.scalar.activation(M2[:], d2[:], Exp)

            # A rows flattened to one partition so per-chunk rank-1 matmuls
            # can read [1, Q] slices at base partition 0
            aflat = sp.tile([1, C * Q], f32, tag="aflat")
            nc.sync.dma_start(aflat[:], A[:])

            # z chunk loads + state contributions, computed transposed:
            # H_T[d, c] = z_c[:, d] . U[:, c]  (two 96-row halves)
            zc = []
            ht_ps = hps.tile([HD, 2 * C], f32, tag="ht")
            for c in range(C):
                t = zp.tile([Q, DBLK], f32, tag="z")
                nc.sync.dma_start(t[:], z_s[c])
                zc.append(t)
                nc.tensor.matmul(
                    ht_ps[:, c : c + 1], t[:, 0:HD], U[:, c : c + 1]
                )
                nc.tensor.matmul(
                    ht_ps[:, C + c : C + c + 1], t[:, HD:DBLK], U[:, c : c + 1]
                )

            ht_sb = sp.tile([HD, 2 * C], f32, tag="ht_sb")
            nc.vector.tensor_copy(ht_sb[:], ht_ps[:])
            # transpose the two halves back to chunk-major H [C, DBLK]
            h_tr = hps.tile([C, DBLK], f32, tag="h_tr")
            nc.tensor.transpose(h_tr[:, 0:HD], ht_sb[:, 0:C], idn[:HD, :HD])
            nc.tensor.transpose(h_tr[:, HD:DBLK], ht_sb[:, C : 2 * C], idn[:HD, :HD])
            H = sp.tile([C, DBLK], f32, tag="H")
            nc.vector.tensor_copy(H[:], h_tr[:])

            c_ps = hps.tile([C, DBLK], f32, tag="cps")
            nc.tensor.matmul(c_ps[:], M2[:], H[:])
            carry = sp.tile([C, DBLK], f32, tag="carry")
            nc.vector.tensor_copy(carry[:], c_ps[:])
            # flatten carry rows to one partition for the rank-1 matmuls
            cflat = sp.tile([1, C * DBLK], f32, tag="cflat")
            nc.sync.dma_start(cflat[:], carry[:])

            # W^T blocks: delta via K=3 matmul, mask, exp (grouped by GRP)
            wT = []
            for g in range(NG):
                dp = dps.tile([Q, GRP * Q], f32, tag="dp")
                for k in range(GRP):
                    c = g * GRP + k
                    nc.tensor.matmul(
                        dp[:, k * Q : (k + 1) * Q],
                        sL[:, c * Q : (c + 1) * Q],
                        sR[:, c * Q : (c + 1) * Q],
                    )
                nc.vector.tensor_add(dp[:], dp[:], mb[:])
                w4 = wp.tile([Q, GRP * Q], f32, tag="w4")
                nc.scalar.activation(w4[:], dp[:], Exp)
                wT.append(w4)

            # outputs: out_c = W_c^T.T @ z_c + A_c (x) carry_c
            for c in range(C):
                g, k = divmod(c, GRP)
                o_ps = ops.tile([Q, DBLK], f32, tag="o")
                nc.tensor.matmul(
                    o_ps[:], wT[g][:, k * Q : (k + 1) * Q], zc[c][:],
                    start=True, stop=False,
                )
                nc.tensor.matmul(
                    o_ps[:],
                    aflat[:, c * Q : (c + 1) * Q],
                    cflat[:, c * DBLK : (c + 1) * DBLK],
                    start=False, stop=True,
                )
                o_sb = op.tile([Q, DBLK], f32, tag="osb")
                if c % 2 == 0:
                    nc.scalar.copy(o_sb[:], o_ps[:])
                else:
                    nc.vector.tensor_copy(o_sb[:], o_ps[:])
                nc.sync.dma_start(out_s[c], o_sb[:])

    nc.compile()
    return nc


def _host_prep(pt_b):
    """Per-batch host-side prep of the small scan operands. pt_b: [L] f32."""
    pt_b = pt_b.astype(np.float64)
    decay = np.clip(1.0 - pt_b, 1e-12, None)
    S = np.cumsum(np.log(decay))
    logp = np.log(np.maximum(pt_b, 1e-38))
    Send = S[Q - 1 :: Q]
    Sendprev = np.concatenate([[0.0], Send[:-1]])

    stackL = np.stack([np.ones(L), -S, logp]).astype(np.float32)
    stackR = np.stack([S, np.ones(L), np.ones(L)]).astype(np.float32)

    Smat = S.reshape(C, Q)
    logpm = logp.reshape(C, Q)
    uexp = (Send[:, None] - Smat + logpm).T.astype(np.float32)
    aexp = (Smat - Sendprev[:, None]).astype(np.float32)
    m_i = np.arange(C)[:, None]
    c_i = np.arange(C)[None, :]
    d2exp = np.where(m_i < c_i, Sendprev[None, :] - Send[:, None], NEG)
    d2exp = d2exp.astype(np.float32)
    return stackL, stackR, uexp, aexp, d2exp


_MASKB = None


def _get_maskb():
    global _MASKB
    if _MASKB is None:
        j = np.arange(Q)[:, None]
        i = np.arange(Q)[None, :]
        one = np.where(i >= j, 0.0, NEG).astype(np.float32)
        _MASKB = np.tile(one, (1, GRP))
    return _MASKB


def _install_ntff_shim():
    """Enable NTFF profiling under axon: shim the missing antenv.axon_hooks
    module and register the ctypes hook from trn_boot; skip the fileshare
    artifact upload (no bucket in this container)."""
    import sys
    import types
    import antenv

    if "antenv.axon_hooks" not in sys.modules:
        mod = types.ModuleType("antenv.axon_hooks")
        hook_box = [None]
        mod.set_axon_ntff_profile_hook = lambda h: hook_box.__setitem__(0, h)
        mod.get_axon_ntff_profile_hook = lambda: hook_box[0]
        mod._hook_box = hook_box
        sys.modules["antenv.axon_hooks"] = mod
        antenv.axon_hooks = mod
    mod = sys.modules["antenv.axon_hooks"]
    if mod.get_axon_ntff_profile_hook() is None:
        from trn_agent_boot.trn_boot import _ntff_profile_via_ctypes

        mod.set_axon_ntff_profile_hook(
            _ntff_profile_via_ctypes("/opt/axon/libaxon_pjrt.so")
        )
    import concourse.bass_utils as bu

    bu.upload_artifacts = lambda tmpdir: f"local://{tmpdir}"


def kernel(z, pt):
    global LAST_EXEC_NS
    from concourse.bass_utils import run_bass_kernel_spmd

    z = np.asarray(z, dtype=np.float32)
    pt = np.asarray(pt, dtype=np.float32)

    if "nc" not in _CTX:
        _CTX["nc"] = _build_program()
    nc = _CTX["nc"]

    maskb = _get_maskb()
    preps = [_host_prep(pt[b]) for b in range(B)]
    in_maps = []
    for core in range(N_CORES):
        b, dblk = divmod(core, ND)
        stackL, stackR, uexp, aexp, d2exp = preps[b]
        z_slab = np.ascontiguousarray(
            z[b, :, dblk * DBLK : (dblk + 1) * DBLK]
        ).reshape(C, Q, DBLK)
        in_maps.append({
            "z_s": z_slab,
            "stackL": stackL,
            "stackR": stackR,
            "uexp": uexp,
            "aexp": aexp,
            "d2exp": d2exp,
            "maskb": maskb,
            "ident": np.eye(Q, dtype=np.float32),
        })

    trace = bool(int(os.environ.get("BASS_KERNEL_TRACE", "0")))
    if trace:
        try:
            _install_ntff_shim()
        except Exception:
            trace = False
    res = run_bass_kernel_spmd(nc, in_maps, list(range(N_CORES)), trace=trace)
    LAST_EXEC_NS = res.exec_time_ns

    out = np.empty((B, L, D), np.float32)
    for core in range(N_CORES):
        b, dblk = divmod(core, ND)
        out[b, :, dblk * DBLK : (dblk + 1) * DBLK] = (
            res.results[core]["out_s"].reshape(L, DBLK)
        )
    return out
